# revision 26
# baseline (speedup 1.0000x reference)
"""Multi-head self-attention on 8 Trainium2 NeuronCores.

Problem: hidden [B=2, S=4096, D=768], H=12 heads x DH=64, fp32.
  q/k/v = x @ Wq/k/v (+bias), per-head softmax(q k^T / 8) @ v, out = ctx @ Wo + bo.

Sharding (per the hint): data-parallel over batch (2) x tensor-parallel over
head groups (4 groups of 3 heads).  Core cid = b*4 + g gets x[b] and the
weight slices for heads [3g, 3g+3) and returns the partial output projection
partial_g = ctx_g @ Wo[192g:192(g+1), :].  The host sums the 4 partials per
batch and adds (bv @ Wo + bo), which is exact because bv/bo enter linearly
(sum_k softmax = 1).  bq/bk are all-zero in this problem (asserted).

Key structure (changes over the first working version):
  * phase 2 is software-pipelined at kc-pair granularity with lag 1:
    the PE emits the two score matmuls for pair g (alternating 64-row
    PE row groups -> the two K=64 matmuls stream concurrently; verified
    ~2x on HW) while the exp of pair g-1 runs on ScalarE/VectorE, then
    the PV matmuls of pair g-1 - the in-order PE queue never
    head-of-line blocks on an exp that hasn't finished.
  * exp is split across two engines, routed per pair: ScalarE
    activation(Exp) for 9 of 16 pairs, VectorE Schraudolph fast-exp
    ((x*A+B) -> int16, bitcast bf16; max rel err ~4%, which averages
    out across the softmax sum and is consistent between numerator and
    denominator since both use the same probs) for the other 7, as one
    fused 1024-wide tensor_scalar per pair.
  * normalization: denominator rows (free via the trailing ones column
    of v_aug) collect into per-head [8,512] tiles, one
    reciprocal_approx_fast per head, then a tiny ones-column PE matmul
    broadcasts each reciprocal row across 64 partitions for the DVE
    multiply.
  * phase 1 is software-pipelined with lag 1 (transposes of s-tile st
    overlap the Q/K/V projection matmuls of st-1), x is converted to
    bf16 so the PE transposes run at 1 cyc/row instead of 2, the second
    128-col chunks of Wq and Wk are packed into one stationary matrix
    so their projections share one matmul chain, and the Q^T/K^T
    row-half duplication DMAs stream per s-tile under the pipeline
    instead of serializing at the phase boundary.
  * PSUM budget: 2x two-bank score pairs + 2 PV banks + 1 broadcast
    bank = 8 banks.

Measured (reps-slope, alternating-order estimator): ~555-567 us vs the
staged baseline's ~568 us under identical conditions.  Rejected via HW
A/B: fp8 DoubleRow PV (per-pair LDWEIGHTS penalty dominates), all-DVE
or DVE-heavy exp routing (sustained DVE tensor_scalar is slower than
ScalarE activation), K=64 split-PV concurrent accumulation into one
PSUM region (runtime failure), GPSIMD partition_broadcast (BIR verify
fails), 0-stride DMA broadcast (rejected: partition step must be
nonzero).
"""

import numpy as np

from contextlib import ExitStack

import concourse.bacc as bacc
import concourse.tile as tile
from concourse import mybir
from concourse.masks import make_identity

F32 = mybir.dt.float32
BF16 = mybir.dt.bfloat16
I16 = mybir.dt.int16

# Full problem constants
B, S, D = 2, 4096, 768
H, DH = 12, 64
N_CORES = 8
GROUPS = 4          # tensor-parallel head groups
HL = H // GROUPS    # heads per core = 3
M = HL * DH         # local projection width = 192
SCALE = 1.0 / float(np.sqrt(DH).astype(np.float32))

P = 128             # partitions
QT = 512            # q tile (free dim of score matmuls)
ACT_PAIRS = (0, 2, 4, 6, 8, 10, 12, 14, 15)  # of 16 kc-pairs/qt -> ScalarE
DVE_FUSED = True    # one 1024-wide DVE exp per pair vs two 512-wide
EVAC_DVE = True     # PV evacuation on VectorE (ScalarE queue is contended)
PR_BUFS = 4
HT_BUFS = 3
PJ_BUFS = 4
TP_BUFS = 3
PHASE2_STUB = False  # ablation: replace phase-2 body with memsets

A16 = 128.0 / np.log(2.0)      # Schraudolph bf16 scale
B16 = 127.0 * 128.0 - 7.4      # Schraudolph bf16 bias


def emit_attention(ctx: ExitStack, tc: tile.TileContext, out_ap, x_ap,
                   wq_ap, wk_ap, wv_ap, wo_ap, s=S, d=D):
    """Emit the per-core attention program.

    out_ap: [s, d] f32    partial output (ctx_local @ Wo_rows)
    x_ap:   [s, d] f32    hidden states for this core's batch
    wq/wk/wv_ap: [d, M]   weight column slices for this core's heads
    wo_ap:  [M, d]        output projection row slice
    """
    nc = tc.nc
    DC = d // P          # d chunks (6)
    NQ = s // QT         # q tiles (8)
    KC = s // P          # k chunks (32)
    SD = s // P          # s chunks (32)
    NPAIR = KC // 2      # kc pairs (16)

    const_pool = ctx.enter_context(tc.tile_pool(name="const", bufs=1))
    identity = const_pool.tile([P, P], F32)
    make_identity(nc, identity)
    ones64 = const_pool.tile([1, 64], BF16)
    nc.vector.memset(ones64, 1.0)

    # ---- persistent activation tiles ----
    qk_pool = ctx.enter_context(tc.tile_pool(name="qk", bufs=1))
    qts = [qk_pool.tile([P, s], BF16, tag=f"qt{h}", name=f"qt{h}") for h in range(HL)]
    kts = [qk_pool.tile([P, s], BF16, tag=f"kt{h}", name=f"kt{h}") for h in range(HL)]
    # V (+ones col) per head, chunked: [128, KC, HL*65] bf16
    v_all = qk_pool.tile([P, KC, HL * 65], BF16)
    for h in range(HL):
        nc.vector.memset(v_all[:, :, h * 65 + 64:h * 65 + 65], 1.0)

    acc_pool = ctx.enter_context(tc.tile_pool(name="acc", bufs=1))
    # unnormalized ctx^T (+denom row 64) per head, bf16
    cx_un = acc_pool.tile([65, HL, s], BF16)
    dns = [acc_pool.tile([NQ, QT], BF16, tag=f"dn{h}", name=f"dn{h}")
           for h in range(HL)]
    dnf = acc_pool.tile([NQ, QT], F32)
    recips = [acc_pool.tile([NQ, QT], F32, tag=f"rc{h}", name=f"rc{h}")
              for h in range(HL)]
    rc_bf = acc_pool.tile([NQ, QT], BF16)

    # ========== phase 1: transpose + Q^T/K^T + V (lag-1 pipeline) ==========
    wb_pool = ctx.enter_context(tc.tile_pool(name="wb", bufs=1))
    wq_t = wb_pool.tile([P, DC, M], BF16)
    wk_t = wb_pool.tile([P, DC, M], BF16)
    wv_t = wb_pool.tile([P, DC, M], BF16)
    wqkB = wb_pool.tile([P, DC, P], BF16)   # [Wq[:,128:192] | Wk[:,128:192]]
    with ExitStack() as p1:
        w1_pool = p1.enter_context(tc.tile_pool(name="w1", bufs=1))
        wq_f = w1_pool.tile([P, DC, M], F32)
        wk_f = w1_pool.tile([P, DC, M], F32)
        wv_f = w1_pool.tile([P, DC, M], F32)
        nc.sync.dma_start(wq_f, wq_ap.rearrange("(c p) m -> p c m", p=P))
        nc.sync.dma_start(wk_f, wk_ap.rearrange("(c p) m -> p c m", p=P))
        nc.sync.dma_start(wv_f, wv_ap.rearrange("(c p) m -> p c m", p=P))
        nc.vector.tensor_copy(wq_t, wq_f)
        nc.vector.tensor_copy(wk_t, wk_f)
        nc.vector.tensor_copy(wv_t, wv_f)
        nc.vector.tensor_copy(wqkB[:, :, 0:64], wq_f[:, :, P:M])
        nc.vector.tensor_copy(wqkB[:, :, 64:P], wk_f[:, :, P:M])

        hn_pool = p1.enter_context(tc.tile_pool(name="hn", bufs=8))
        hb_pool = p1.enter_context(tc.tile_pool(name="hb", bufs=8))
        ht_pool = p1.enter_context(tc.tile_pool(name="ht", bufs=HT_BUFS))
        tp_psum = p1.enter_context(
            tc.tile_pool(name="tp", bufs=TP_BUFS, space="PSUM"))
        pj_psum = p1.enter_context(
            tc.tile_pool(name="pj", bufs=PJ_BUFS, space="PSUM"))

        identity_bf = const_pool.tile([P, P], BF16)
        nc.vector.tensor_copy(identity_bf, identity)

        hns = {}
        hts = {}

        def load_x(st):
            tl = []
            for j in range(4):
                hn = hn_pool.tile([P, d], F32, tag="hn", name=f"hn{st}_{j}")
                nc.sync.dma_start(
                    hn, x_ap[(st * 4 + j) * P:(st * 4 + j + 1) * P, :])
                hb = hb_pool.tile([P, d], BF16, tag="hb", name=f"hb{st}_{j}")
                nc.scalar.copy(hb, hn)
                tl.append(hb)
            hns[st] = tl

        def transpose_x(st):
            # bf16 transposes run at 1 cyc/row on the PE vs 2 for fp32
            ht = ht_pool.tile([P, DC, QT], BF16, tag="ht", name="ht")
            for c in range(DC):
                tp = tp_psum.tile([P, QT], BF16, tag="tp", name="tp")
                for j in range(4):
                    nc.tensor.transpose(tp[:, j * P:(j + 1) * P],
                                        hns[st][j][:, c * P:(c + 1) * P],
                                        identity_bf)
                nc.scalar.copy(ht[:, c, :], tp)
            hts[st] = ht

        def project(st):
            ss = slice(st * QT, (st + 1) * QT)
            ht = hts.pop(st)
            # Q chunk A (heads 0,1), K chunk A, then packed Q/K chunk B
            for (w_t, dsts) in ((wq_t, qts), (wk_t, kts)):
                psA = pj_psum.tile([P, QT], F32, tag="pj", name="psA")
                for c in range(DC):
                    nc.tensor.matmul(psA, lhsT=w_t[:, c, 0:P], rhs=ht[:, c, :],
                                     start=(c == 0), stop=(c == DC - 1))
                nc.vector.tensor_copy(dsts[0][0:64, ss], psA[0:64, :])
                nc.vector.tensor_copy(dsts[1][64:P, ss], psA[64:P, :])
            psB = pj_psum.tile([P, QT], F32, tag="pj", name="psB")
            for c in range(DC):
                nc.tensor.matmul(psB, lhsT=wqkB[:, c, :], rhs=ht[:, c, :],
                                 start=(c == 0), stop=(c == DC - 1))
            nc.vector.tensor_copy(qts[2][0:64, ss], psB[0:64, :])
            nc.vector.tensor_copy(kts[2][64:P, ss], psB[64:P, :])
            # duplicate this s-slice of each Q^T/K^T into the missing row
            # half right away (DMA, overlapped under the phase-1 pipeline)
            nc.sync.dma_start(qts[0][64:P, ss], qts[0][0:64, ss])
            nc.sync.dma_start(qts[1][0:64, ss], qts[1][64:P, ss])
            nc.sync.dma_start(qts[2][64:P, ss], qts[2][0:64, ss])
            nc.sync.dma_start(kts[0][64:P, ss], kts[0][0:64, ss])
            nc.sync.dma_start(kts[1][0:64, ss], kts[1][64:P, ss])
            nc.sync.dma_start(kts[2][0:64, ss], kts[2][64:P, ss])
            # V natural for the 4 s-subchunks
            for j in range(4):
                psV = pj_psum.tile([P, QT], F32, tag="pj", name="psV")
                for c in range(DC):
                    nc.tensor.matmul(psV[:, 0:M],
                                     lhsT=ht[:, c, j * P:(j + 1) * P],
                                     rhs=wv_t[:, c, :],
                                     start=(c == 0), stop=(c == DC - 1))
                ic = st * 4 + j
                for h in range(HL):
                    nc.vector.tensor_copy(v_all[:, ic, h * 65:h * 65 + 64],
                                          psV[:, h * DH:(h + 1) * DH])
            del hns[st]

        load_x(0)
        for st in range(NQ):
            if st + 1 < NQ:
                load_x(st + 1)
            transpose_x(st)
            if st > 0:
                project(st - 1)
        project(NQ - 1)

    # ================= phase 2: attention (lag-1 pair pipeline) =============
    ctx_pool = ctx.enter_context(tc.tile_pool(name="ctxn", bufs=1))
    ctxn_a = ctx_pool.tile([P, s], BF16)      # head 0 (rows 0:64), head 1 staged in
    ctxn_h1 = ctx_pool.tile([64, s], BF16)    # head 1 staging at rows 0:64
    ctxn_b = ctx_pool.tile([64, s], BF16)     # head 2

    with ExitStack() as p2:
        sc_psum = p2.enter_context(
            tc.tile_pool(name="sc", bufs=2, space="PSUM"))
        pv_psum = p2.enter_context(
            tc.tile_pool(name="pv", bufs=2, space="PSUM"))
        bc_psum = p2.enter_context(
            tc.tile_pool(name="bc", bufs=1, space="PSUM"))
        pr_pool = p2.enter_context(tc.tile_pool(name="pr", bufs=PR_BUFS))
        st_pool = p2.enter_context(tc.tile_pool(name="stg", bufs=3))

        pv_tiles = {}

        def emit_scores(h, qt_i, g):
            qs = slice(qt_i * QT, (qt_i + 1) * QT)
            sc = sc_psum.tile([P, 2, QT], F32, tag="sc", name="sc")
            for j in range(2):
                kc = 2 * g + j
                half = (kc % 2) * 64
                nc.tensor.matmul(
                    sc[:, j, :],
                    lhsT=kts[h][half:half + 64, kc * P:(kc + 1) * P],
                    rhs=qts[h][half:half + 64, qs],
                    start=True, stop=True)
            return sc

        def emit_exp_pv(h, qt_i, g, sc):
            if (h, qt_i) not in pv_tiles:
                pv_tiles[(h, qt_i)] = pv_psum.tile([P, QT], F32, tag="pv",
                                                   name="pv")
            pv = pv_tiles[(h, qt_i)]
            pr = pr_pool.tile([P, 2, QT], BF16, tag="pr", name="pr")
            if g in ACT_PAIRS:
                for j in range(2):
                    nc.scalar.activation(pr[:, j, :], sc[:, j, :],
                                         mybir.ActivationFunctionType.Exp,
                                         scale=SCALE)
            elif DVE_FUSED:
                nc.vector.tensor_scalar(
                    pr.bitcast(I16), sc,
                    A16 * SCALE, B16,
                    mybir.AluOpType.mult, mybir.AluOpType.add)
            else:
                for j in range(2):
                    nc.vector.tensor_scalar(
                        pr[:, j, :].bitcast(I16), sc[:, j, :],
                        A16 * SCALE, B16,
                        mybir.AluOpType.mult, mybir.AluOpType.add)
            for j in range(2):
                kc = 2 * g + j
                nc.tensor.matmul(
                    pv[0:65, :],
                    lhsT=v_all[:, kc, h * 65:(h + 1) * 65],
                    rhs=pr[:, j, :],
                    start=(kc == 0), stop=(kc == KC - 1))
            if g == NPAIR - 1:
                finish_qt(h, qt_i)

        def finish_qt(h, qt_i):
            qs = slice(qt_i * QT, (qt_i + 1) * QT)
            pv = pv_tiles.pop((h, qt_i))
            if EVAC_DVE:
                nc.vector.tensor_copy(cx_un[:, h, qs], pv[0:65, :])
            else:
                nc.scalar.copy(cx_un[:, h, qs], pv[0:65, :])
            nc.sync.dma_start(dns[h][qt_i:qt_i + 1, :], cx_un[64:65, h, qs])
            if qt_i == NQ - 1:
                normalize_head(h)

        def normalize_head(h):
            nc.vector.tensor_copy(dnf, dns[h])
            nc.vector.reciprocal_approx_fast(recips[h], dnf)
            nc.vector.tensor_copy(rc_bf, recips[h])
            for qt_i in range(NQ):
                qs = slice(qt_i * QT, (qt_i + 1) * QT)
                stg = st_pool.tile([1, QT], BF16, tag="stg", name="stg")
                nc.sync.dma_start(stg, rc_bf[qt_i:qt_i + 1, :])
                bc = bc_psum.tile([64, QT], F32, tag="bc", name="bc")
                nc.tensor.matmul(bc, lhsT=ones64, rhs=stg, start=True,
                                 stop=True)
                dst = (ctxn_a[0:64, qs], ctxn_h1[:, qs], ctxn_b[:, qs])[h]
                nc.vector.tensor_tensor(dst, cx_un[0:64, h, qs], bc,
                                        mybir.AluOpType.mult)
            if h == 1:
                nc.sync.dma_start(ctxn_a[64:P, :], ctxn_h1[:, :])

        if PHASE2_STUB:
            nc.vector.memset(cx_un, 0.5)
            for h in range(HL):
                nc.vector.memset(dns[h], 1000.0)
                normalize_head(h)
        else:
            pending = []
            for h in range(HL):
                for qt_i in range(NQ):
                    for g in range(NPAIR):
                        sc = emit_scores(h, qt_i, g)
                        pending.append((h, qt_i, g, sc))
                        if len(pending) > 1:
                            emit_exp_pv(*pending.pop(0))
            while pending:
                emit_exp_pv(*pending.pop(0))

    # ================= phase 3: output projection =================
    with ExitStack() as p4:
        w3_pool = p4.enter_context(tc.tile_pool(name="w3", bufs=1))
        wo_af = w3_pool.tile([P, d], F32)
        wo_bf = w3_pool.tile([64, d], F32)
        nc.sync.dma_start(wo_af, wo_ap[0:P, :])
        nc.sync.dma_start(wo_bf, wo_ap[P:M, :])
        wo_a = w3_pool.tile([P, d], BF16)
        wo_b = w3_pool.tile([64, d], BF16)
        nc.vector.tensor_copy(wo_a, wo_af)
        nc.vector.tensor_copy(wo_b, wo_bf)
        op_psum = p4.enter_context(
            tc.tile_pool(name="op", bufs=3, space="PSUM"))
        ob_pool = p4.enter_context(tc.tile_pool(name="ob", bufs=3))
        ntiles = [(i * QT, min(QT, d - i * QT)) for i in range((d + QT - 1) // QT)]
        for si in range(SD):
            ssl = slice(si * P, (si + 1) * P)
            ot = ob_pool.tile([P, d], F32, tag="ot", name="ot")
            # chain-outer order: each ctxn stationary chunk is loaded once
            # and reused across both n-tiles
            ops = [op_psum.tile([P, QT], F32, tag="op", name="op")
                   for _ in ntiles]
            for (n0, nw), op in zip(ntiles, ops):
                nc.tensor.matmul(op[:, 0:nw], lhsT=ctxn_a[:, ssl],
                                 rhs=wo_a[:, n0:n0 + nw], start=True, stop=False)
            for (n0, nw), op in zip(ntiles, ops):
                nc.tensor.matmul(op[:, 0:nw], lhsT=ctxn_b[:, ssl],
                                 rhs=wo_b[:, n0:n0 + nw], start=False, stop=True)
                nc.scalar.copy(ot[:, n0:n0 + nw], op[:, 0:nw])
            nc.sync.dma_start(out_ap[ssl, :], ot)


def build_program(s=S, d=D, reps=1):
    nc = bacc.Bacc("TRN2", target_bir_lowering=False, debug=False,
                   enable_asserts=False, num_devices=N_CORES)
    x_t = nc.dram_tensor("x", [s, d], F32, kind="ExternalInput")
    wq_t = nc.dram_tensor("wq", [d, M], F32, kind="ExternalInput")
    wk_t = nc.dram_tensor("wk", [d, M], F32, kind="ExternalInput")
    wv_t = nc.dram_tensor("wv", [d, M], F32, kind="ExternalInput")
    wo_t = nc.dram_tensor("wo", [M, d], F32, kind="ExternalInput")
    out_t = nc.dram_tensor("out", [s, d], F32, kind="ExternalOutput")
    with tile.TileContext(nc) as tc:
        for _ in range(reps):
            with ExitStack() as ctx:
                emit_attention(ctx, tc, out_t.ap(), x_t.ap(), wq_t.ap(),
                               wk_t.ap(), wv_t.ap(), wo_t.ap(), s=s, d=d)
    nc.compile()
    return nc


_NC_CACHE = {}


def kernel(hidden_states, Wq, bq, Wk, bk, Wv, bv, Wo, bo):
    from concourse.bass_utils import run_bass_kernel_spmd

    hidden_states = np.asarray(hidden_states, dtype=np.float32)
    Wq, Wk, Wv, Wo = (np.asarray(w, dtype=np.float32) for w in (Wq, Wk, Wv, Wo))
    bq, bk, bv, bo = (np.asarray(b_, dtype=np.float32) for b_ in (bq, bk, bv, bo))
    assert float(np.abs(bq).max(initial=0.0)) == 0.0, "nonzero bq unsupported"
    assert float(np.abs(bk).max(initial=0.0)) == 0.0, "nonzero bk unsupported"

    if "nc" not in _NC_CACHE:
        _NC_CACHE["nc"] = build_program()
    nc = _NC_CACHE["nc"]

    in_maps = []
    for cid in range(N_CORES):
        b_i, g = divmod(cid, GROUPS)
        ms = slice(g * M, (g + 1) * M)
        in_maps.append({
            "x": np.ascontiguousarray(hidden_states[b_i]),
            "wq": np.ascontiguousarray(Wq[:, ms]),
            "wk": np.ascontiguousarray(Wk[:, ms]),
            "wv": np.ascontiguousarray(Wv[:, ms]),
            "wo": np.ascontiguousarray(Wo[ms, :]),
        })
    res = run_bass_kernel_spmd(nc, in_maps, core_ids=list(range(N_CORES)))
    # bv and bo enter linearly: ctx = ctx0 + bv  =>  out += bv @ Wo + bo
    host_bias = (bv @ Wo + bo).astype(np.float32)
    out = np.empty((B, S, D), dtype=np.float32)
    for b_i in range(B):
        acc = res.results[b_i * GROUPS + 0]["out"].astype(np.float32)
        for g in range(1, GROUPS):
            acc = acc + res.results[b_i * GROUPS + g]["out"]
        out[b_i] = acc + host_bias
    return out


# revision 27
# speedup vs baseline: 1.0177x; 1.0177x over previous
"""Multi-head self-attention on 8 Trainium2 NeuronCores.

Problem: hidden [B=2, S=4096, D=768], H=12 heads x DH=64, fp32.
  q/k/v = x @ Wq/k/v (+bias), per-head softmax(q k^T / 8) @ v, out = ctx @ Wo + bo.

Sharding (per the hint): data-parallel over batch (2) x tensor-parallel over
head groups (4 groups of 3 heads).  Core cid = b*4 + g gets x[b] and the
weight slices for heads [3g, 3g+3) and returns the partial output projection
partial_g = ctx_g @ Wo[192g:192(g+1), :].  The host sums the 4 partials per
batch and adds (bv @ Wo + bo), which is exact because bv/bo enter linearly
(sum_k softmax = 1).  bq/bk are all-zero in this problem (asserted).

Key structure (changes over the first working version):
  * phase 2 is software-pipelined at kc-pair granularity with lag 1:
    the PE emits the two score matmuls for pair g (alternating 64-row
    PE row groups -> the two K=64 matmuls stream concurrently; verified
    ~2x on HW) while the exp of pair g-1 runs on ScalarE/VectorE, then
    the PV matmuls of pair g-1 - the in-order PE queue never
    head-of-line blocks on an exp that hasn't finished.
  * exp is split across two engines, routed per pair: ScalarE
    activation(Exp) for 9 of 16 pairs, VectorE Schraudolph fast-exp
    ((x*A+B) -> int16, bitcast bf16; max rel err ~4%, which averages
    out across the softmax sum and is consistent between numerator and
    denominator since both use the same probs) for the other 7, as one
    fused 1024-wide tensor_scalar per pair.
  * normalization: denominator rows (free via the trailing ones column
    of v_aug) collect into per-head [8,512] tiles, one
    reciprocal_approx_fast per head, then a tiny ones-column PE matmul
    broadcasts each reciprocal row across 64 partitions for the DVE
    multiply.
  * phase 1 is software-pipelined with lag 1 (transposes of s-tile st
    overlap the Q/K/V projection matmuls of st-1), x is converted to
    bf16 so the PE transposes run at 1 cyc/row instead of 2, the second
    128-col chunks of Wq and Wk are packed into one stationary matrix
    so their projections share one matmul chain, and the Q^T/K^T
    row-half duplication DMAs stream per s-tile under the pipeline
    instead of serializing at the phase boundary.
  * PSUM budget: 2x two-bank score pairs + 2 PV banks + 1 broadcast
    bank = 8 banks.

Measured (reps-slope, alternating-order estimator): ~555-567 us vs the
staged baseline's ~568 us under identical conditions.  Rejected via HW
A/B: fp8 DoubleRow PV (per-pair LDWEIGHTS penalty dominates), all-DVE
or DVE-heavy exp routing (sustained DVE tensor_scalar is slower than
ScalarE activation), K=64 split-PV concurrent accumulation into one
PSUM region (runtime failure), GPSIMD partition_broadcast (BIR verify
fails), 0-stride DMA broadcast (rejected: partition step must be
nonzero).
"""

import numpy as np

from contextlib import ExitStack

import concourse.bacc as bacc
import concourse.tile as tile
from concourse import mybir
from concourse.masks import make_identity

F32 = mybir.dt.float32
BF16 = mybir.dt.bfloat16
I16 = mybir.dt.int16

# Full problem constants
B, S, D = 2, 4096, 768
H, DH = 12, 64
N_CORES = 8
GROUPS = 4          # tensor-parallel head groups
HL = H // GROUPS    # heads per core = 3
M = HL * DH         # local projection width = 192
SCALE = 1.0 / float(np.sqrt(DH).astype(np.float32))

P = 128             # partitions
QT = 512            # q tile (free dim of score matmuls)
ACT_PAIRS = (0, 2, 4, 6, 8, 10, 12, 14, 15)  # of 16 kc-pairs/qt -> ScalarE
DVE_FUSED = True    # one 1024-wide DVE exp per pair vs two 512-wide
EVAC_DVE = True     # PV evacuation on VectorE (ScalarE queue is contended)
PR_BUFS = 4
HT_BUFS = 3
PJ_BUFS = 4
TP_BUFS = 3
PHASE2_STUB = False  # ablation: replace phase-2 body with memsets

A16 = 128.0 / np.log(2.0)      # Schraudolph bf16 scale
B16 = 127.0 * 128.0 - 7.4      # Schraudolph bf16 bias


def emit_attention(ctx: ExitStack, tc: tile.TileContext, out_ap, x_ap,
                   wq_ap, wk_ap, wv_ap, wo_ap, s=S, d=D):
    """Emit the per-core attention program.

    out_ap: [s, d] f32    partial output (ctx_local @ Wo_rows)
    x_ap:   [s, d] f32    hidden states for this core's batch
    wq/wk/wv_ap: [d, M]   weight column slices for this core's heads
    wo_ap:  [M, d]        output projection row slice
    """
    nc = tc.nc
    DC = d // P          # d chunks (6)
    NQ = s // QT         # q tiles (8)
    KC = s // P          # k chunks (32)
    SD = s // P          # s chunks (32)
    NPAIR = KC // 2      # kc pairs (16)

    const_pool = ctx.enter_context(tc.tile_pool(name="const", bufs=1))
    identity = const_pool.tile([P, P], F32)
    make_identity(nc, identity)
    ones64 = const_pool.tile([1, 64], BF16)
    nc.vector.memset(ones64, 1.0)

    # ---- persistent activation tiles ----
    qk_pool = ctx.enter_context(tc.tile_pool(name="qk", bufs=1))
    qts = [qk_pool.tile([P, s], BF16, tag=f"qt{h}", name=f"qt{h}") for h in range(HL)]
    kts = [qk_pool.tile([P, s], BF16, tag=f"kt{h}", name=f"kt{h}") for h in range(HL)]
    # V (+ones col) per head, chunked: [128, KC, HL*65] bf16
    v_all = qk_pool.tile([P, KC, HL * 65], BF16)
    for h in range(HL):
        nc.vector.memset(v_all[:, :, h * 65 + 64:h * 65 + 65], 1.0)

    acc_pool = ctx.enter_context(tc.tile_pool(name="acc", bufs=1))
    # unnormalized ctx^T (+denom row 64) per head, bf16
    cx_un = acc_pool.tile([65, HL, s], BF16)
    dns = [acc_pool.tile([NQ, QT], BF16, tag=f"dn{h}", name=f"dn{h}")
           for h in range(HL)]
    dnf = acc_pool.tile([NQ, QT], F32)
    recips = [acc_pool.tile([NQ, QT], F32, tag=f"rc{h}", name=f"rc{h}")
              for h in range(HL)]
    rc_bf = acc_pool.tile([NQ, QT], BF16)

    # ========== phase 1: transpose + Q^T/K^T + V (lag-1 pipeline) ==========
    wb_pool = ctx.enter_context(tc.tile_pool(name="wb", bufs=1))
    wq_t = wb_pool.tile([P, DC, M], BF16)
    wk_t = wb_pool.tile([P, DC, M], BF16)
    wv_t = wb_pool.tile([P, DC, M], BF16)
    wqkB = wb_pool.tile([P, DC, P], BF16)   # [Wq[:,128:192] | Wk[:,128:192]]
    with ExitStack() as p1:
        w1_pool = p1.enter_context(tc.tile_pool(name="w1", bufs=1))
        wq_f = w1_pool.tile([P, DC, M], F32)
        wk_f = w1_pool.tile([P, DC, M], F32)
        wv_f = w1_pool.tile([P, DC, M], F32)
        nc.sync.dma_start(wq_f, wq_ap.rearrange("(c p) m -> p c m", p=P))
        nc.sync.dma_start(wk_f, wk_ap.rearrange("(c p) m -> p c m", p=P))
        nc.sync.dma_start(wv_f, wv_ap.rearrange("(c p) m -> p c m", p=P))
        nc.vector.tensor_copy(wq_t, wq_f)
        nc.vector.tensor_copy(wk_t, wk_f)
        nc.vector.tensor_copy(wv_t, wv_f)
        nc.vector.tensor_copy(wqkB[:, :, 0:64], wq_f[:, :, P:M])
        nc.vector.tensor_copy(wqkB[:, :, 64:P], wk_f[:, :, P:M])

        hn_pool = p1.enter_context(tc.tile_pool(name="hn", bufs=8))
        hb_pool = p1.enter_context(tc.tile_pool(name="hb", bufs=8))
        ht_pool = p1.enter_context(tc.tile_pool(name="ht", bufs=HT_BUFS))
        tp_psum = p1.enter_context(
            tc.tile_pool(name="tp", bufs=TP_BUFS, space="PSUM"))
        pj_psum = p1.enter_context(
            tc.tile_pool(name="pj", bufs=PJ_BUFS, space="PSUM"))

        identity_bf = const_pool.tile([P, P], BF16)
        nc.vector.tensor_copy(identity_bf, identity)

        hns = {}
        hts = {}

        def load_x(st):
            tl = []
            for j in range(4):
                hn = hn_pool.tile([P, d], F32, tag="hn", name=f"hn{st}_{j}")
                nc.sync.dma_start(
                    hn, x_ap[(st * 4 + j) * P:(st * 4 + j + 1) * P, :])
                hb = hb_pool.tile([P, d], BF16, tag="hb", name=f"hb{st}_{j}")
                nc.scalar.copy(hb, hn)
                tl.append(hb)
            hns[st] = tl

        def transpose_x(st):
            # bf16 transposes run at 1 cyc/row on the PE vs 2 for fp32
            ht = ht_pool.tile([P, DC, QT], BF16, tag="ht", name="ht")
            for c in range(DC):
                tp = tp_psum.tile([P, QT], BF16, tag="tp", name="tp")
                for j in range(4):
                    nc.tensor.transpose(tp[:, j * P:(j + 1) * P],
                                        hns[st][j][:, c * P:(c + 1) * P],
                                        identity_bf)
                nc.scalar.copy(ht[:, c, :], tp)
            hts[st] = ht

        def project(st):
            ss = slice(st * QT, (st + 1) * QT)
            ht = hts.pop(st)
            # Q chunk A (heads 0,1), K chunk A, then packed Q/K chunk B
            for (w_t, dsts) in ((wq_t, qts), (wk_t, kts)):
                psA = pj_psum.tile([P, QT], F32, tag="pj", name="psA")
                for c in range(DC):
                    nc.tensor.matmul(psA, lhsT=w_t[:, c, 0:P], rhs=ht[:, c, :],
                                     start=(c == 0), stop=(c == DC - 1))
                nc.vector.tensor_copy(dsts[0][0:64, ss], psA[0:64, :])
                nc.vector.tensor_copy(dsts[1][64:P, ss], psA[64:P, :])
            psB = pj_psum.tile([P, QT], F32, tag="pj", name="psB")
            for c in range(DC):
                nc.tensor.matmul(psB, lhsT=wqkB[:, c, :], rhs=ht[:, c, :],
                                 start=(c == 0), stop=(c == DC - 1))
            nc.vector.tensor_copy(qts[2][0:64, ss], psB[0:64, :])
            nc.vector.tensor_copy(kts[2][64:P, ss], psB[64:P, :])
            # duplicate this s-slice of each Q^T/K^T into the missing row
            # half right away (DMA, overlapped under the phase-1 pipeline)
            nc.sync.dma_start(qts[0][64:P, ss], qts[0][0:64, ss])
            nc.sync.dma_start(qts[1][0:64, ss], qts[1][64:P, ss])
            nc.sync.dma_start(qts[2][64:P, ss], qts[2][0:64, ss])
            nc.sync.dma_start(kts[0][64:P, ss], kts[0][0:64, ss])
            nc.sync.dma_start(kts[1][0:64, ss], kts[1][64:P, ss])
            nc.sync.dma_start(kts[2][0:64, ss], kts[2][64:P, ss])
            # V natural for the 4 s-subchunks
            for j in range(4):
                psV = pj_psum.tile([P, QT], F32, tag="pj", name="psV")
                for c in range(DC):
                    nc.tensor.matmul(psV[:, 0:M],
                                     lhsT=ht[:, c, j * P:(j + 1) * P],
                                     rhs=wv_t[:, c, :],
                                     start=(c == 0), stop=(c == DC - 1))
                ic = st * 4 + j
                # one strided copy for all 3 heads (dst skips the ones cols)
                nc.vector.tensor_copy(
                    v_all[:, ic, :].rearrange("p (h x) -> p h x", x=65)[:, :, 0:DH],
                    psV[:, 0:M].rearrange("p (h x) -> p h x", x=DH))
            del hns[st]

        load_x(0)
        for st in range(NQ):
            if st + 1 < NQ:
                load_x(st + 1)
            transpose_x(st)
            if st > 0:
                project(st - 1)
        project(NQ - 1)

    # ================= phase 2: attention (lag-1 pair pipeline) =============
    ctx_pool = ctx.enter_context(tc.tile_pool(name="ctxn", bufs=1))
    ctxn_a = ctx_pool.tile([P, s], BF16)      # head 0 (rows 0:64), head 1 staged in
    ctxn_h1 = ctx_pool.tile([64, s], BF16)    # head 1 staging at rows 0:64
    ctxn_b = ctx_pool.tile([64, s], BF16)     # head 2

    with ExitStack() as p2:
        sc_psum = p2.enter_context(
            tc.tile_pool(name="sc", bufs=2, space="PSUM"))
        pv_psum = p2.enter_context(
            tc.tile_pool(name="pv", bufs=2, space="PSUM"))
        bc_psum = p2.enter_context(
            tc.tile_pool(name="bc", bufs=1, space="PSUM"))
        pr_pool = p2.enter_context(tc.tile_pool(name="pr", bufs=PR_BUFS))
        st_pool = p2.enter_context(tc.tile_pool(name="stg", bufs=3))

        pv_tiles = {}

        def emit_scores(h, qt_i, g):
            qs = slice(qt_i * QT, (qt_i + 1) * QT)
            sc = sc_psum.tile([P, 2, QT], F32, tag="sc", name="sc")
            for j in range(2):
                kc = 2 * g + j
                half = (kc % 2) * 64
                nc.tensor.matmul(
                    sc[:, j, :],
                    lhsT=kts[h][half:half + 64, kc * P:(kc + 1) * P],
                    rhs=qts[h][half:half + 64, qs],
                    start=True, stop=True)
            return sc

        def emit_exp_pv(h, qt_i, g, sc):
            if (h, qt_i) not in pv_tiles:
                pv_tiles[(h, qt_i)] = pv_psum.tile([P, QT], F32, tag="pv",
                                                   name="pv")
            pv = pv_tiles[(h, qt_i)]
            pr = pr_pool.tile([P, 2, QT], BF16, tag="pr", name="pr")
            if g in ACT_PAIRS:
                for j in range(2):
                    nc.scalar.activation(pr[:, j, :], sc[:, j, :],
                                         mybir.ActivationFunctionType.Exp,
                                         scale=SCALE)
            elif DVE_FUSED:
                nc.vector.tensor_scalar(
                    pr.bitcast(I16), sc,
                    A16 * SCALE, B16,
                    mybir.AluOpType.mult, mybir.AluOpType.add)
            else:
                for j in range(2):
                    nc.vector.tensor_scalar(
                        pr[:, j, :].bitcast(I16), sc[:, j, :],
                        A16 * SCALE, B16,
                        mybir.AluOpType.mult, mybir.AluOpType.add)
            for j in range(2):
                kc = 2 * g + j
                nc.tensor.matmul(
                    pv[0:65, :],
                    lhsT=v_all[:, kc, h * 65:(h + 1) * 65],
                    rhs=pr[:, j, :],
                    start=(kc == 0), stop=(kc == KC - 1))
            if g == NPAIR - 1:
                finish_qt(h, qt_i)

        def finish_qt(h, qt_i):
            qs = slice(qt_i * QT, (qt_i + 1) * QT)
            pv = pv_tiles.pop((h, qt_i))
            if EVAC_DVE:
                nc.vector.tensor_copy(cx_un[:, h, qs], pv[0:65, :])
            else:
                nc.scalar.copy(cx_un[:, h, qs], pv[0:65, :])
            nc.sync.dma_start(dns[h][qt_i:qt_i + 1, :], cx_un[64:65, h, qs])
            if qt_i == NQ - 1:
                normalize_head(h)

        def normalize_head(h):
            nc.vector.tensor_copy(dnf, dns[h])
            nc.vector.reciprocal_approx_fast(recips[h], dnf)
            nc.vector.tensor_copy(rc_bf, recips[h])
            for qt_i in range(NQ):
                qs = slice(qt_i * QT, (qt_i + 1) * QT)
                stg = st_pool.tile([1, QT], BF16, tag="stg", name="stg")
                nc.sync.dma_start(stg, rc_bf[qt_i:qt_i + 1, :])
                bc = bc_psum.tile([64, QT], F32, tag="bc", name="bc")
                nc.tensor.matmul(bc, lhsT=ones64, rhs=stg, start=True,
                                 stop=True)
                dst = (ctxn_a[0:64, qs], ctxn_h1[:, qs], ctxn_b[:, qs])[h]
                nc.vector.tensor_tensor(dst, cx_un[0:64, h, qs], bc,
                                        mybir.AluOpType.mult)
            if h == 1:
                nc.sync.dma_start(ctxn_a[64:P, :], ctxn_h1[:, :])

        if PHASE2_STUB:
            nc.vector.memset(cx_un, 0.5)
            for h in range(HL):
                nc.vector.memset(dns[h], 1000.0)
                normalize_head(h)
        else:
            pending = []
            for h in range(HL):
                for qt_i in range(NQ):
                    for g in range(NPAIR):
                        sc = emit_scores(h, qt_i, g)
                        pending.append((h, qt_i, g, sc))
                        if len(pending) > 1:
                            emit_exp_pv(*pending.pop(0))
            while pending:
                emit_exp_pv(*pending.pop(0))

    # ================= phase 3: output projection =================
    with ExitStack() as p4:
        w3_pool = p4.enter_context(tc.tile_pool(name="w3", bufs=1))
        wo_af = w3_pool.tile([P, d], F32)
        wo_bf = w3_pool.tile([64, d], F32)
        nc.sync.dma_start(wo_af, wo_ap[0:P, :])
        nc.sync.dma_start(wo_bf, wo_ap[P:M, :])
        wo_a = w3_pool.tile([P, d], BF16)
        wo_b = w3_pool.tile([64, d], BF16)
        nc.vector.tensor_copy(wo_a, wo_af)
        nc.vector.tensor_copy(wo_b, wo_bf)
        op_psum = p4.enter_context(
            tc.tile_pool(name="op", bufs=3, space="PSUM"))
        ob_pool = p4.enter_context(tc.tile_pool(name="ob", bufs=3))
        ntiles = [(i * QT, min(QT, d - i * QT)) for i in range((d + QT - 1) // QT)]
        for si in range(SD):
            ssl = slice(si * P, (si + 1) * P)
            ot = ob_pool.tile([P, d], F32, tag="ot", name="ot")
            # chain-outer order: each ctxn stationary chunk is loaded once
            # and reused across both n-tiles
            ops = [op_psum.tile([P, QT], F32, tag="op", name="op")
                   for _ in ntiles]
            for (n0, nw), op in zip(ntiles, ops):
                nc.tensor.matmul(op[:, 0:nw], lhsT=ctxn_a[:, ssl],
                                 rhs=wo_a[:, n0:n0 + nw], start=True, stop=False)
            for (n0, nw), op in zip(ntiles, ops):
                nc.tensor.matmul(op[:, 0:nw], lhsT=ctxn_b[:, ssl],
                                 rhs=wo_b[:, n0:n0 + nw], start=False, stop=True)
                nc.scalar.copy(ot[:, n0:n0 + nw], op[:, 0:nw])
            nc.sync.dma_start(out_ap[ssl, :], ot)


def build_program(s=S, d=D, reps=1):
    nc = bacc.Bacc("TRN2", target_bir_lowering=False, debug=False,
                   enable_asserts=False, num_devices=N_CORES)
    x_t = nc.dram_tensor("x", [s, d], F32, kind="ExternalInput")
    wq_t = nc.dram_tensor("wq", [d, M], F32, kind="ExternalInput")
    wk_t = nc.dram_tensor("wk", [d, M], F32, kind="ExternalInput")
    wv_t = nc.dram_tensor("wv", [d, M], F32, kind="ExternalInput")
    wo_t = nc.dram_tensor("wo", [M, d], F32, kind="ExternalInput")
    out_t = nc.dram_tensor("out", [s, d], F32, kind="ExternalOutput")
    with tile.TileContext(nc) as tc:
        for _ in range(reps):
            with ExitStack() as ctx:
                emit_attention(ctx, tc, out_t.ap(), x_t.ap(), wq_t.ap(),
                               wk_t.ap(), wv_t.ap(), wo_t.ap(), s=s, d=d)
    nc.compile()
    return nc


_NC_CACHE = {}


def kernel(hidden_states, Wq, bq, Wk, bk, Wv, bv, Wo, bo):
    from concourse.bass_utils import run_bass_kernel_spmd

    hidden_states = np.asarray(hidden_states, dtype=np.float32)
    Wq, Wk, Wv, Wo = (np.asarray(w, dtype=np.float32) for w in (Wq, Wk, Wv, Wo))
    bq, bk, bv, bo = (np.asarray(b_, dtype=np.float32) for b_ in (bq, bk, bv, bo))
    assert float(np.abs(bq).max(initial=0.0)) == 0.0, "nonzero bq unsupported"
    assert float(np.abs(bk).max(initial=0.0)) == 0.0, "nonzero bk unsupported"

    if "nc" not in _NC_CACHE:
        _NC_CACHE["nc"] = build_program()
    nc = _NC_CACHE["nc"]

    in_maps = []
    for cid in range(N_CORES):
        b_i, g = divmod(cid, GROUPS)
        ms = slice(g * M, (g + 1) * M)
        in_maps.append({
            "x": np.ascontiguousarray(hidden_states[b_i]),
            "wq": np.ascontiguousarray(Wq[:, ms]),
            "wk": np.ascontiguousarray(Wk[:, ms]),
            "wv": np.ascontiguousarray(Wv[:, ms]),
            "wo": np.ascontiguousarray(Wo[ms, :]),
        })
    res = run_bass_kernel_spmd(nc, in_maps, core_ids=list(range(N_CORES)))
    # bv and bo enter linearly: ctx = ctx0 + bv  =>  out += bv @ Wo + bo
    host_bias = (bv @ Wo + bo).astype(np.float32)
    out = np.empty((B, S, D), dtype=np.float32)
    for b_i in range(B):
        acc = res.results[b_i * GROUPS + 0]["out"].astype(np.float32)
        for g in range(1, GROUPS):
            acc = acc + res.results[b_i * GROUPS + g]["out"]
        out[b_i] = acc + host_bias
    return out
